# revision 1
# baseline (speedup 1.0000x reference)
"""Trainium2 Bass kernel for nn_MultiHeadAttention (B=8, S=1024, HID=1024, NH=16).

Data-parallel over batch across 8 NeuronCores (1 batch element/core).
Per-core pipeline (single Tile program):
  1. PE-transpose activations (XqT, XkT fp32; Xv bf16 streamed) and weights
     (Wq*8/Wk fp32 streamed per o-block; Wv/Wo bf16 full).
  2. Projections on PE: qT = 8*(Wq@XqT+bq) etc (fp32 matmuls + K=1 bias-row
     pass); qT/kT split hi/lo bf16 -> QH/QL/KH/KL. v natural bf16, masked by
     K_mask and augmented with [K_mask, 1] columns -> VM.
  3. Per (head, q-block): scores psum [128,1024] via 3 bf16 matmuls
     (hh+hl+lh, err ~2^-17); DVE rowmax (negated); ACT exp(s-max) -> e bf16;
     PE-transpose e -> eT -> ET chunk.
  4. ctx^T psum [66,512] = sum_kb VM' @ ET; rows 64/65 = U = sum(e*Km),
     D = sum(e).
  5. scale = Q_mask/(U + S*1e-8*D) batched; ctx *= scale; out = ctx@WoT+bo.
"""

import numpy as np

import concourse.bass as bass
import concourse.tile as tile
from concourse import bacc, mybir
from concourse.bass_utils import run_bass_kernel_spmd
from concourse.masks import make_identity

F32 = mybir.dt.float32
BF16 = mybir.dt.bfloat16
AF = mybir.ActivationFunctionType
ALU = mybir.AluOpType
AX = mybir.AxisListType

B, S, HID, NH, DH = 8, 1024, 1024, 16, 64
P = 128
NP = HID // P        # 8 hid partition-blocks
NSB = S // P         # 8 seq blocks
NPAIR = NH // 2
CREN = float(S) * 1e-8

_CACHE = {}

import os
_STAGE = int(os.environ.get("KSTAGE", "3"))


def _build():
    nc = bacc.Bacc("TRN2", target_bir_lowering=False, debug=False, num_devices=B)

    Xq = nc.dram_tensor("Xq", [S, HID], F32, kind="ExternalInput").ap()
    Xk = nc.dram_tensor("Xk", [S, HID], F32, kind="ExternalInput").ap()
    Xv = nc.dram_tensor("Xv", [S, HID], F32, kind="ExternalInput").ap()
    Qm = nc.dram_tensor("Qm", [S], F32, kind="ExternalInput").ap()
    Km = nc.dram_tensor("Km", [S], F32, kind="ExternalInput").ap()
    Wq = nc.dram_tensor("Wq", [HID, HID], F32, kind="ExternalInput").ap()
    Wk = nc.dram_tensor("Wk", [HID, HID], F32, kind="ExternalInput").ap()
    Wv = nc.dram_tensor("Wv", [HID, HID], F32, kind="ExternalInput").ap()
    Wo = nc.dram_tensor("Wo", [HID, HID], F32, kind="ExternalInput").ap()
    bqv = nc.dram_tensor("bq", [HID], F32, kind="ExternalInput").ap()
    bkv = nc.dram_tensor("bk", [HID], F32, kind="ExternalInput").ap()
    bvv = nc.dram_tensor("bv", [HID], F32, kind="ExternalInput").ap()
    bov = nc.dram_tensor("bo", [HID], F32, kind="ExternalInput").ap()
    out = nc.dram_tensor("out", [S, HID], F32, kind="ExternalOutput").ap()
    scl_dram = nc.dram_tensor("scl_scratch", [32, 512], F32).ap()

    with tile.TileContext(nc) as tc:
        with (
            tc.tile_pool(name="consts", bufs=1) as consts,
            tc.tile_pool(name="xrows", bufs=2) as xrows,
            tc.tile_pool(name="bigx", bufs=1) as bigx,     # XqT->XkT->WvT->WoT
            tc.tile_pool(name="wc", bufs=2) as wcp,        # streamed W^T o-chunks
            tc.tile_pool(name="xvc", bufs=2) as xvcp,      # streamed XvT chunks
            tc.tile_pool(name="qk", bufs=1) as qkp,
            tc.tile_pool(name="vm", bufs=1) as vmp,
            tc.tile_pool(name="et", bufs=1) as etp,
            tc.tile_pool(name="epool", bufs=2) as epool,
            tc.tile_pool(name="ctx", bufs=1) as ctxp,
            tc.tile_pool(name="smalls", bufs=4) as smalls,
            tc.tile_pool(name="stg", bufs=2) as stg,
            tc.tile_pool(name="scdup", bufs=2) as scdup,
            tc.tile_pool(name="ps_big", bufs=2, space="PSUM") as ps_big,
            tc.tile_pool(name="ps_mid", bufs=2, space="PSUM") as ps_mid,
            tc.tile_pool(name="ps_sm", bufs=2, space="PSUM") as ps_sm,
        ):
            # ---------------- constants ----------------
            idf = consts.tile([P, P], F32, name="idf")
            make_identity(nc, idf[:])
            idb = consts.tile([P, P], BF16, name="idb")
            nc.vector.tensor_copy(idb[:], idf[:])
            # matmul operands need base partition in {0,32,64}
            rowA = consts.tile([65, HID], F32, name="rowA")  # bq8@0, bk@32, bv@64
            rowB = consts.tile([65, HID], F32, name="rowB")  # bo@0, Qm@64
            onesr = consts.tile([65, 512], F32, name="onesr")  # ones at 0/32/64
            nc.vector.memset(onesr[0:1, :], 1.0)
            nc.vector.memset(onesr[32:33, :], 1.0)
            nc.vector.memset(onesr[64:65, :], 1.0)
            nc.sync.dma_start(rowA[0:1, :], bqv[None, :])
            nc.vector.tensor_scalar_mul(rowA[0:1, :], rowA[0:1, :], 8.0)
            nc.sync.dma_start(rowA[32:33, :], bkv[None, :])
            nc.sync.dma_start(rowA[64:65, :], bvv[None, :])
            nc.sync.dma_start(rowB[0:1, :], bov[None, :])
            nc.sync.dma_start(rowB[64:65, :], Qm[None, :])
            km_pi = consts.tile([P, NSB], F32, name="km_pi")
            nc.sync.dma_start(km_pi[:], Km.rearrange("(o p) -> p o", p=P))

            # ------------- helper: transpose X -> [P, NP, S] -------------
            def build_xt(x_dram, dst, dtype):
                for sb in range(NSB):
                    xr = xrows.tile([P, HID], F32, tag="xr32")
                    nc.sync.dma_start(xr[:], x_dram[sb * P:(sb + 1) * P, :])
                    if dtype == BF16:
                        xb = xrows.tile([P, HID], BF16, tag="xr16")
                        nc.vector.tensor_copy(xb[:], xr[:])
                        xr, ident, pool, ptag = xb, idb, ps_sm, "smb"
                    else:
                        ident, pool, ptag = idf, ps_mid, "mid"
                    for g in range(2):
                        pt = pool.tile([P, 512], dtype, tag=ptag)
                        for t in range(4):
                            ib = g * 4 + t
                            nc.tensor.transpose(pt[:, t * P:(t + 1) * P],
                                                xr[:, ib * P:(ib + 1) * P],
                                                ident[:])
                        nc.scalar.activation(
                            dst[:, g * 4:(g + 1) * 4, sb * P:(sb + 1) * P],
                            pt[:].rearrange("p (t c) -> p t c", t=4), AF.Copy)

            # ------- helper: transpose one o-block of W -> [P, NP, P] -------
            def build_wchunk(w_dram, ob, dtype, scale=None):
                tag = "wc32" if dtype == F32 else "wc16"
                wch = wcp.tile([P, NP, P], dtype, tag=tag)
                xr = xrows.tile([P, HID], F32, tag="xr32")
                nc.sync.dma_start(xr[:], w_dram[ob * P:(ob + 1) * P, :])
                if dtype == BF16:
                    xb = xrows.tile([P, HID], BF16, tag="xr16")
                    nc.vector.tensor_copy(xb[:], xr[:])
                    xr, ident, pool, ptag = xb, idb, ps_sm, "smb"
                else:
                    ident, pool, ptag = idf, ps_mid, "mid"
                for g in range(2):
                    pt = pool.tile([P, 512], dtype, tag=ptag)
                    for t in range(4):
                        ib = g * 4 + t
                        nc.tensor.transpose(pt[:, t * P:(t + 1) * P],
                                            xr[:, ib * P:(ib + 1) * P], ident[:])
                    dst = wch[:, g * 4:(g + 1) * 4, :]
                    src = pt[:].rearrange("p (t c) -> p t c", t=4)
                    if scale is None:
                        nc.scalar.activation(dst, src, AF.Copy)
                    else:
                        nc.vector.tensor_scalar_mul(dst, src, scale)
                return wch

            # ---------------- q/k projections (fp32) ----------------
            QH = qkp.tile([P, NP, S], BF16, name="QH")
            QL = qkp.tile([P, NP, S], BF16, name="QL")
            KH = qkp.tile([P, NP, S], BF16, name="KH")
            KL = qkp.tile([P, NP, S], BF16, name="KL")

            def qk_projection(x_dram, w_dram, brow, ones1, hi, lo, scale):
                xt_full = bigx.tile([P, NP, S], F32, tag="bigx")
                build_xt(x_dram, xt_full, F32)
                for ob in range(NP):
                    wch = build_wchunk(w_dram, ob, F32, scale=scale)
                    for sc in range(2):
                        ss = slice(sc * 512, (sc + 1) * 512)
                        pp = ps_mid.tile([P, 512], F32, tag="mid")
                        for m in range(NP):
                            nc.tensor.matmul(pp[:], wch[:, m, :],
                                             xt_full[:, m, ss],
                                             start=(m == 0), stop=False)
                        nc.tensor.matmul(pp[:], brow[:, ob * P:(ob + 1) * P],
                                         ones1[:, 0:512], start=False, stop=True)
                        hs = hi[:, ob, ss]
                        nc.scalar.activation(hs, pp[:], AF.Copy)
                        nc.vector.scalar_tensor_tensor(
                            lo[:, ob, ss], pp[:], 1.0, hs,
                            ALU.mult, ALU.subtract)

            qk_projection(Xq, Wq, rowA[0:1, :], onesr[0:1, :], QH, QL, 8.0)
            qk_projection(Xk, Wk, rowA[32:33, :], onesr[32:33, :], KH, KL, None)

            # ---------------- v projection (bf16) ----------------
            VM = vmp.tile([P, NSB, NH, 66], BF16, name="VM")
            WvT = bigx.tile([P, NP, HID], BF16, tag="bigx")
            for ob in range(NP):
                wch = build_wchunk(Wv, ob, BF16)
                nc.vector.tensor_copy(WvT[:, :, ob * P:(ob + 1) * P], wch[:])
            for scq in range(4):
                xvc = xvcp.tile([P, NP, 256], BF16, tag="xvc")
                for sb2 in range(2):
                    sb = scq * 2 + sb2
                    xr = xrows.tile([P, HID], F32, tag="xr32")
                    nc.sync.dma_start(xr[:], Xv[sb * P:(sb + 1) * P, :])
                    xb = xrows.tile([P, HID], BF16, tag="xr16")
                    nc.vector.tensor_copy(xb[:], xr[:])
                    for g in range(2):
                        pt = ps_sm.tile([P, 512], BF16, tag="smb")
                        for t in range(4):
                            ib = g * 4 + t
                            nc.tensor.transpose(pt[:, t * P:(t + 1) * P],
                                                xb[:, ib * P:(ib + 1) * P],
                                                idb[:])
                        nc.scalar.activation(
                            xvc[:, g * 4:(g + 1) * 4, sb2 * P:(sb2 + 1) * P],
                            pt[:].rearrange("p (t c) -> p t c", t=4), AF.Copy)
                for sb2 in range(2):
                    sb = scq * 2 + sb2
                    for oc in range(2):
                        pp = ps_mid.tile([P, 512], F32, tag="mid")
                        for m in range(NP):
                            nc.tensor.matmul(
                                pp[:], xvc[:, m, sb2 * P:(sb2 + 1) * P],
                                WvT[:, m, oc * 512:(oc + 1) * 512],
                                start=(m == 0), stop=False)
                        nc.tensor.matmul(pp[:], onesr[64:65, 0:P],
                                         rowA[64:65, oc * 512:(oc + 1) * 512],
                                         start=False, stop=True)
                        nc.vector.tensor_scalar_mul(
                            VM[:, sb, oc * 8:(oc + 1) * 8, 0:64],
                            pp[:].rearrange("p (h d) -> p h d", h=8),
                            km_pi[:, sb:sb + 1])
            for sb in range(NSB):
                nc.vector.tensor_copy(
                    VM[:, sb, :, 64:65],
                    km_pi[:, sb:sb + 1, None].to_broadcast([P, NH, 1]))
            nc.vector.memset(VM[:, :, :, 65:66], 1.0)

            if _STAGE == 1:
                dbg = stg.tile([P, 512], F32, tag="ot")
                nc.vector.tensor_copy(dbg[:], QH[:, 0, 0:512])
                nc.sync.dma_start(out[0:P, 0:512], dbg[:])
                dbg2 = stg.tile([P, 512], F32, tag="ot")
                nc.vector.tensor_copy(dbg2[:], KH[:, 0, 0:512])
                nc.sync.dma_start(out[0:P, 512:1024], dbg2[:])

            if _STAGE >= 2:
                # ---------------- attention ----------------
                ST = consts.tile([32, S], F32, name="ST")
                ctxu = ctxp.tile([P, NPAIR, S], BF16, name="ctxu")

                for h in range(NH):
                    j, pb = h // 2, 64 * (h % 2)
                    for qc in range(2):
                        et_c = etp.tile([P, NSB, 512], BF16, tag="etc")
                        for qb4 in range(4):
                            qb = qc * 4 + qb4
                            sp = ps_big.tile([P, S], F32, tag="big")
                            qs = slice(qb * P, (qb + 1) * P)
                            for kc in range(2):
                                ks = slice(kc * 512, (kc + 1) * 512)
                                nc.tensor.matmul(sp[:, ks], QH[pb:pb + 64, j, qs],
                                                 KH[pb:pb + 64, j, ks],
                                                 start=True, stop=False)
                                nc.tensor.matmul(sp[:, ks], QH[pb:pb + 64, j, qs],
                                                 KL[pb:pb + 64, j, ks],
                                                 start=False, stop=False)
                                nc.tensor.matmul(sp[:, ks], QL[pb:pb + 64, j, qs],
                                                 KH[pb:pb + 64, j, ks],
                                                 start=False, stop=True)
                            nmax = smalls.tile([P, 1], F32, tag="nmax")
                            nc.vector.tensor_reduce(nmax[:], sp[:], axis=AX.X,
                                                    op=ALU.max, negate=True)
                            e_t = epool.tile([P, S], BF16, tag="e")
                            nc.scalar.activation(e_t[:], sp[:], AF.Exp,
                                                 bias=nmax[:], scale=1.0)
                            ep = ps_sm.tile([P, S], BF16, tag="smb")
                            for kb in range(NSB):
                                nc.tensor.transpose(ep[:, kb * P:(kb + 1) * P],
                                                    e_t[:, kb * P:(kb + 1) * P],
                                                    idb[:])
                            nc.vector.tensor_copy(
                                et_c[:, :, qb4 * P:(qb4 + 1) * P],
                                ep[:].rearrange("p (kb q) -> p kb q", kb=NSB))
                        cp = ps_mid.tile([P, 512], F32, tag="mid")
                        for kb in range(NSB):
                            nc.tensor.matmul(cp[0:66, :], VM[:, kb, h, :],
                                             et_c[:, kb, :],
                                             start=(kb == 0), stop=(kb == NSB - 1))
                        idx = h * 2 + qc
                        ud = stg.tile([2, 512], F32, tag="ud")
                        nc.vector.tensor_copy(ud[:], cp[64:66, :])
                        nc.sync.dma_start(ST[idx:idx + 1, 0:512], ud[0:1, :])
                        nc.sync.dma_start(ST[idx:idx + 1, 512:1024], ud[1:2, :])
                        if pb == 0:
                            nc.vector.tensor_copy(
                                ctxu[0:64, j, qc * 512:(qc + 1) * 512], cp[0:64, :])
                        else:
                            sg = stg.tile([64, 512], BF16, tag="stg")
                            nc.vector.tensor_copy(sg[:], cp[0:64, :])
                            nc.sync.dma_start(
                                ctxu[64:128, j, qc * 512:(qc + 1) * 512], sg[:])

                if _STAGE == 2:
                    for j in range(NPAIR):
                        dbg = stg.tile([P, 512], F32, tag="ot")
                        nc.vector.tensor_copy(dbg[:], ctxu[:, j, 0:512])
                        nc.sync.dma_start(out[j * P:(j + 1) * P, 0:512], dbg[:])

            if _STAGE >= 3 and _STAGE != 27:
                # ---------------- batched renorm ----------------
                qmst = consts.tile([32, 512], F32, name="qmst")
                for idx in range(32):
                    qc = idx % 2
                    nc.sync.dma_start(qmst[idx:idx + 1, :],
                                      rowB[64:65, qc * 512:(qc + 1) * 512])
                scl = consts.tile([32, 512], F32, name="scl")
                # scl = Qm / (U + CREN * D);  U = ST[:, 0:512], D = ST[:, 512:]
                nc.vector.scalar_tensor_tensor(scl[:], ST[:, 512:1024], CREN,
                                               ST[:, 0:512], ALU.mult, ALU.add)
                nc.vector.reciprocal(scl[:], scl[:])
                nc.vector.tensor_tensor(scl[:], scl[:], qmst[:], ALU.mult)
                nc.sync.dma_start(scl_dram[:], scl[:])

                for j in range(NPAIR):
                    for qc in range(2):
                        sd = scdup.tile([P, 512], F32, tag="sd")
                        ia = (2 * j) * 2 + qc
                        ibx = (2 * j + 1) * 2 + qc
                        nc.sync.dma_start(
                            sd[0:64, :], scl_dram[ia:ia + 1, :].to_broadcast([64, 512]))
                        nc.sync.dma_start(
                            sd[64:128, :],
                            scl_dram[ibx:ibx + 1, :].to_broadcast([64, 512]))
                        nc.vector.tensor_tensor(
                            ctxu[:, j, qc * 512:(qc + 1) * 512],
                            ctxu[:, j, qc * 512:(qc + 1) * 512], sd[:], ALU.mult)

                if _STAGE == 25:
                    for j in range(NPAIR):
                        dbg = stg.tile([P, 512], F32, tag="ot")
                        nc.vector.tensor_copy(dbg[:], ctxu[:, j, 0:512])
                        nc.sync.dma_start(out[j * P:(j + 1) * P, 0:512], dbg[:])

            if _STAGE >= 3 and _STAGE != 25:
                # ---------------- output projection ----------------
                # _STAGE 28: no bias pass; 29: even heads only (+bias)
                WoT = bigx.tile([P, NP, HID], BF16, tag="bigx")
                for ob in range(NP):
                    wch = build_wchunk(Wo, ob, BF16)
                    nc.vector.tensor_copy(WoT[:, :, ob * P:(ob + 1) * P], wch[:])
                for qb in range(NSB):
                    for oc in range(2):
                        op_ = ps_mid.tile([P, 512], F32, tag="mid")
                        for j in range(NPAIR):
                            nc.tensor.matmul(
                                op_[:], ctxu[:, j, qb * P:(qb + 1) * P],
                                WoT[:, j, oc * 512:(oc + 1) * 512],
                                start=(j == 0), stop=False)
                        nc.tensor.matmul(op_[:], onesr[0:1, 0:P],
                                         rowB[0:1, oc * 512:(oc + 1) * 512],
                                         start=False, stop=True)
                        ot = stg.tile([P, 512], F32, tag="ot")
                        nc.vector.tensor_copy(ot[:], op_[:])
                        nc.sync.dma_start(
                            out[qb * P:(qb + 1) * P, oc * 512:(oc + 1) * 512], ot[:])

    nc.compile()
    return nc


def kernel(Q, K, V, Q_mask, K_mask, Wq, bq, Wk, bk, Wv, bv, Wo, bo):
    if "nc" not in _CACHE:
        _CACHE["nc"] = _build()
    nc = _CACHE["nc"]
    Q = np.ascontiguousarray(np.asarray(Q, np.float32))
    K = np.ascontiguousarray(np.asarray(K, np.float32))
    V = np.ascontiguousarray(np.asarray(V, np.float32))
    shared = {
        "Wq": np.ascontiguousarray(np.asarray(Wq, np.float32)),
        "Wk": np.ascontiguousarray(np.asarray(Wk, np.float32)),
        "Wv": np.ascontiguousarray(np.asarray(Wv, np.float32)),
        "Wo": np.ascontiguousarray(np.asarray(Wo, np.float32)),
        "bq": np.ascontiguousarray(np.asarray(bq, np.float32)),
        "bk": np.ascontiguousarray(np.asarray(bk, np.float32)),
        "bv": np.ascontiguousarray(np.asarray(bv, np.float32)),
        "bo": np.ascontiguousarray(np.asarray(bo, np.float32)),
    }
    in_maps = []
    for i in range(B):
        m = dict(shared)
        m["Xq"] = np.ascontiguousarray(Q[i])
        m["Xk"] = np.ascontiguousarray(K[i])
        m["Xv"] = np.ascontiguousarray(V[i])
        m["Qm"] = np.ascontiguousarray(np.asarray(Q_mask[i], np.float32))
        m["Km"] = np.ascontiguousarray(np.asarray(K_mask[i], np.float32))
        in_maps.append(m)
    res = run_bass_kernel_spmd(nc, in_maps, list(range(B)))
    return np.stack([res.results[i]["out"] for i in range(B)], axis=0)



# revision 3
# speedup vs baseline: 1.4184x; 1.4184x over previous
"""Trainium2 Bass kernel for nn_MultiHeadAttention (B=8, S=1024, HID=1024, NH=16).

Data-parallel over batch across 8 NeuronCores (1 batch element/core).
Host prep: weights pre-transposed (WqT pre-scaled by 8, bq by 8), activations
pre-transposed; Q/K path fp32r, V/O path bf16.

Per-core pipeline (single Tile program):
  1. qT/kT = WT @ XT as fp32r matmuls (1 cyc/row); bias added via ACT
     Identity (per-partition AP bias) during PSUM->SBUF copy; output f32r.
  2. v natural [seq,feat] via bf16 matmuls + K=1 bias row; masked by K_mask
     and augmented with [K_mask, 1] columns -> VM bf16.
  3. Per (head, q-block): scores psum [128,1024] via 2 fp32r matmuls (K=64);
     DVE rowmax (negated); ACT exp(s-max) -> e bf16 SBUF; DMA-xbar transpose
     e -> ET chunk (no PE, no DVE).
  4. ctx^T psum [66,512] = sum_kb VM' @ ET; rows 64/65 = U = sum(e*Km),
     D = sum(e).
  5. scale = Q_mask/(U + S*1e-8*D) batched; ctx *= scale; out = ctx@WoT+bo.
"""

import numpy as np
import ml_dtypes

import concourse.bass as bass
import concourse.tile as tile
from concourse import bacc, mybir
from concourse.bass_utils import run_bass_kernel_spmd

F32 = mybir.dt.float32
F32R = mybir.dt.float32r
BF16 = mybir.dt.bfloat16
AF = mybir.ActivationFunctionType
ALU = mybir.AluOpType
AX = mybir.AxisListType

B, S, HID, NH, DH = 8, 1024, 1024, 16, 64
P = 128
NP = HID // P        # 8 hid partition-blocks
NSB = S // P         # 8 seq blocks
NPAIR = NH // 2
CREN = float(S) * 1e-8

_CACHE = {}


def _build():
    nc = bacc.Bacc("TRN2", target_bir_lowering=False, debug=False, num_devices=B)

    XqT = nc.dram_tensor("XqT", [HID, S], F32R, kind="ExternalInput").ap()
    XkT = nc.dram_tensor("XkT", [HID, S], F32R, kind="ExternalInput").ap()
    XvT = nc.dram_tensor("XvT", [HID, S], BF16, kind="ExternalInput").ap()
    Qm = nc.dram_tensor("Qm", [S], F32, kind="ExternalInput").ap()
    Km = nc.dram_tensor("Km", [S], F32, kind="ExternalInput").ap()
    WqT = nc.dram_tensor("WqT", [HID, HID], F32R, kind="ExternalInput").ap()
    WkT = nc.dram_tensor("WkT", [HID, HID], F32R, kind="ExternalInput").ap()
    WvT = nc.dram_tensor("WvT", [HID, HID], BF16, kind="ExternalInput").ap()
    WoT = nc.dram_tensor("WoT", [HID, HID], BF16, kind="ExternalInput").ap()
    bq8 = nc.dram_tensor("bq8", [HID], F32, kind="ExternalInput").ap()
    bkv = nc.dram_tensor("bk", [HID], F32, kind="ExternalInput").ap()
    bvb = nc.dram_tensor("bvb", [HID], BF16, kind="ExternalInput").ap()
    bob = nc.dram_tensor("bob", [HID], BF16, kind="ExternalInput").ap()
    out = nc.dram_tensor("out", [S, HID], F32, kind="ExternalOutput").ap()
    scl_dram = nc.dram_tensor("scl_scratch", [32, 512], F32).ap()

    with tile.TileContext(nc) as tc:
        with (
            tc.tile_pool(name="consts", bufs=1) as consts,
            tc.tile_pool(name="xh", bufs=3) as xh,          # X^T halves
            tc.tile_pool(name="wch", bufs=2) as wch,        # Wq/Wk^T ob-chunks
            tc.tile_pool(name="wbf", bufs=1) as wbf,        # WvT -> WoT bf16
            tc.tile_pool(name="qk", bufs=1) as qkp,
            tc.tile_pool(name="vm", bufs=1) as vmp,
            tc.tile_pool(name="et", bufs=2) as etp,
            tc.tile_pool(name="epool", bufs=2) as epool,
            tc.tile_pool(name="ctx", bufs=1) as ctxp,
            tc.tile_pool(name="smalls", bufs=4) as smalls,
            tc.tile_pool(name="ud", bufs=1) as udp,
            tc.tile_pool(name="pf512", bufs=2) as pf512,
            tc.tile_pool(name="ps_big", bufs=3, space="PSUM") as ps_big,
            tc.tile_pool(name="ps_mid", bufs=2, space="PSUM") as ps_mid,
        ):
            # ---------------- constants ----------------
            km_pi = consts.tile([P, NSB], F32, name="km_pi")
            nc.sync.dma_start(km_pi[:], Km.rearrange("(o p) -> p o", p=P))
            bq8p = consts.tile([P, NP], F32, name="bq8p")
            nc.sync.dma_start(bq8p[:], bq8.rearrange("(o p) -> p o", p=P))
            bkp = consts.tile([P, NP], F32, name="bkp")
            nc.sync.dma_start(bkp[:], bkv.rearrange("(o p) -> p o", p=P))
            onesb = consts.tile([1, P], BF16, name="onesb")
            nc.vector.memset(onesb[:], 1.0)
            bvrow = consts.tile([1, HID], BF16, name="bvrow")
            nc.sync.dma_start(bvrow[:], bvb[None, :])
            borow = consts.tile([1, HID], BF16, name="borow")
            nc.sync.dma_start(borow[:], bob[None, :])
            ST = consts.tile([32, S], F32, name="ST")

            # ---------------- q/k projections (fp32r) ----------------
            qt = qkp.tile([P, NP, S], F32R, name="qt")
            kt = qkp.tile([P, NP, S], F32R, name="kt")

            def qk_projection(x_dram, w_dram, biasp, dst):
                xhalves = []
                for sc in range(2):
                    xt = xh.tile([P, NP, 512], F32R, tag="xh")
                    nc.sync.dma_start(
                        xt[:],
                        x_dram.rearrange("(m p) s -> p m s", p=P)
                        [:, :, sc * 512:(sc + 1) * 512])
                    xhalves.append(xt)
                for ob in range(NP):
                    wc = wch.tile([P, NP, P], F32R, tag="wch")
                    nc.sync.dma_start(
                        wc[:],
                        w_dram.rearrange("(m p) f -> p m f", p=P)
                        [:, :, ob * P:(ob + 1) * P])
                    for sc in range(2):
                        pp = ps_mid.tile([P, 512], F32, tag="mid")
                        for m in range(NP):
                            nc.tensor.matmul(pp[:], wc[:, m, :],
                                             xhalves[sc][:, m, :],
                                             start=(m == 0), stop=(m == NP - 1))
                        nc.scalar.activation(
                            dst[:, ob, sc * 512:(sc + 1) * 512], pp[:],
                            AF.Identity, bias=biasp[:, ob:ob + 1], scale=1.0)

            qk_projection(XqT, WqT, bq8p, qt)
            qk_projection(XkT, WkT, bkp, kt)

            # ---------------- v projection (bf16, natural layout) ----------
            VM = vmp.tile([P, NSB, NH, 66], BF16, name="VM")
            wvt = wbf.tile([P, NP, HID], BF16, tag="wbf")
            nc.sync.dma_start(wvt[:], WvT.rearrange("(m p) f -> p m f", p=P))
            for sc in range(2):
                xvt = xh.tile([P, NP, 512], BF16, tag="xh")
                nc.sync.dma_start(
                    xvt[:],
                    XvT.rearrange("(m p) s -> p m s", p=P)
                    [:, :, sc * 512:(sc + 1) * 512])
                for sb2 in range(4):
                    sb = sc * 4 + sb2
                    for oc in range(2):
                        pp = ps_mid.tile([P, 512], F32, tag="mid")
                        for m in range(NP):
                            nc.tensor.matmul(
                                pp[:], xvt[:, m, sb2 * P:(sb2 + 1) * P],
                                wvt[:, m, oc * 512:(oc + 1) * 512],
                                start=(m == 0), stop=False)
                        nc.tensor.matmul(pp[:], onesb[:],
                                         bvrow[:, oc * 512:(oc + 1) * 512],
                                         start=False, stop=True)
                        nc.vector.tensor_scalar_mul(
                            VM[:, sb, oc * 8:(oc + 1) * 8, 0:64],
                            pp[:].rearrange("p (h d) -> p h d", h=8),
                            km_pi[:, sb:sb + 1])
            for sb in range(NSB):
                nc.vector.tensor_copy(
                    VM[:, sb, :, 64:65],
                    km_pi[:, sb:sb + 1, None].to_broadcast([P, NH, 1]))
            nc.vector.memset(VM[:, :, :, 65:66], 1.0)

            # ---------------- attention ----------------
            ctxu = ctxp.tile([P, NPAIR, S], BF16, name="ctxu")

            for h in range(NH):
                j, pb = h // 2, 64 * (h % 2)
                for qc in range(2):
                    et_c = etp.tile([P, NSB, 512], BF16, tag="etc")
                    for qb4 in range(4):
                        qb = qc * 4 + qb4
                        sp = ps_big.tile([P, S], F32, tag="big")
                        qs = slice(qb * P, (qb + 1) * P)
                        for kc in range(2):
                            ks = slice(kc * 512, (kc + 1) * 512)
                            nc.tensor.matmul(sp[:, ks], qt[pb:pb + 64, j, qs],
                                             kt[pb:pb + 64, j, ks],
                                             start=True, stop=True)
                        nmax = smalls.tile([P, 1], F32, tag="nmax")
                        nc.vector.tensor_reduce(nmax[:], sp[:], axis=AX.X,
                                                op=ALU.max, negate=True)
                        e_t = epool.tile([P, S], BF16, tag="e")
                        nc.scalar.activation(e_t[:], sp[:], AF.Exp,
                                             bias=nmax[:], scale=1.0)
                        nc.sync.dma_start(
                            et_c[:, :, qb4 * P:(qb4 + 1) * P], e_t[:],
                            transpose=True)
                    cp = ps_mid.tile([P, 512], F32, tag="mid")
                    for kb in range(NSB):
                        nc.tensor.matmul(cp[0:66, :], VM[:, kb, h, :],
                                         et_c[:, kb, :],
                                         start=(kb == 0), stop=(kb == NSB - 1))
                    idx = h * 2 + qc
                    ud = udp.tile([2, 512], F32, tag="ud")
                    nc.vector.tensor_copy(ud[:], cp[64:66, :])
                    nc.sync.dma_start(ST[idx:idx + 1, 0:512], ud[0:1, :])
                    nc.sync.dma_start(ST[idx:idx + 1, 512:1024], ud[1:2, :])
                    if pb == 0:
                        nc.vector.tensor_copy(
                            ctxu[0:64, j, qc * 512:(qc + 1) * 512], cp[0:64, :])
                    else:
                        sg = pf512.tile([64, 512], BF16, tag="pf512")
                        nc.vector.tensor_copy(sg[:], cp[0:64, :])
                        nc.sync.dma_start(
                            ctxu[64:128, j, qc * 512:(qc + 1) * 512], sg[:])

            # ---------------- batched renorm ----------------
            qmst = consts.tile([32, 512], F32, name="qmst")
            for idx in range(32):
                qc = idx % 2
                nc.sync.dma_start(qmst[idx:idx + 1, :],
                                  Qm[None, qc * 512:(qc + 1) * 512])
            scl = consts.tile([32, 512], F32, name="scl")
            # scl = Qm / (U + CREN * D);  U = ST[:, 0:512], D = ST[:, 512:]
            nc.vector.scalar_tensor_tensor(scl[:], ST[:, 512:1024], CREN,
                                           ST[:, 0:512], ALU.mult, ALU.add)
            nc.vector.reciprocal(scl[:], scl[:])
            nc.vector.tensor_tensor(scl[:], scl[:], qmst[:], ALU.mult)
            nc.sync.dma_start(scl_dram[:], scl[:])

            for j in range(NPAIR):
                for qc in range(2):
                    sd = pf512.tile([P, 512], F32, tag="pf512")
                    ia = (2 * j) * 2 + qc
                    ibx = (2 * j + 1) * 2 + qc
                    nc.sync.dma_start(
                        sd[0:64, :],
                        scl_dram[ia:ia + 1, :].to_broadcast([64, 512]))
                    nc.sync.dma_start(
                        sd[64:128, :],
                        scl_dram[ibx:ibx + 1, :].to_broadcast([64, 512]))
                    nc.vector.tensor_tensor(
                        ctxu[:, j, qc * 512:(qc + 1) * 512],
                        ctxu[:, j, qc * 512:(qc + 1) * 512], sd[:], ALU.mult)

            # ---------------- output projection (bf16) ----------------
            wot = wbf.tile([P, NP, HID], BF16, tag="wbf")
            nc.sync.dma_start(wot[:], WoT.rearrange("(m p) f -> p m f", p=P))
            for qb in range(NSB):
                for oc in range(2):
                    op_ = ps_mid.tile([P, 512], F32, tag="mid")
                    for j in range(NPAIR):
                        nc.tensor.matmul(
                            op_[:], ctxu[:, j, qb * P:(qb + 1) * P],
                            wot[:, j, oc * 512:(oc + 1) * 512],
                            start=(j == 0), stop=False)
                    nc.tensor.matmul(op_[:], onesb[:],
                                     borow[:, oc * 512:(oc + 1) * 512],
                                     start=False, stop=True)
                    ot = pf512.tile([P, 512], F32, tag="pf512")
                    nc.scalar.activation(ot[:], op_[:], AF.Copy)
                    nc.sync.dma_start(
                        out[qb * P:(qb + 1) * P, oc * 512:(oc + 1) * 512], ot[:])

    nc.compile()
    return nc


def kernel(Q, K, V, Q_mask, K_mask, Wq, bq, Wk, bk, Wv, bv, Wo, bo):
    if "nc" not in _CACHE:
        _CACHE["nc"] = _build()
    nc = _CACHE["nc"]
    Q = np.asarray(Q, np.float32)
    K = np.asarray(K, np.float32)
    V = np.asarray(V, np.float32)
    bf = ml_dtypes.bfloat16
    shared = {
        "WqT": np.ascontiguousarray((8.0 * np.asarray(Wq, np.float32)).T),
        "WkT": np.ascontiguousarray(np.asarray(Wk, np.float32).T),
        "WvT": np.ascontiguousarray(np.asarray(Wv, np.float32).T.astype(bf)),
        "WoT": np.ascontiguousarray(np.asarray(Wo, np.float32).T.astype(bf)),
        "bq8": np.ascontiguousarray(8.0 * np.asarray(bq, np.float32)),
        "bk": np.ascontiguousarray(np.asarray(bk, np.float32)),
        "bvb": np.ascontiguousarray(np.asarray(bv, np.float32).astype(bf)),
        "bob": np.ascontiguousarray(np.asarray(bo, np.float32).astype(bf)),
    }
    in_maps = []
    for i in range(B):
        m = dict(shared)
        m["XqT"] = np.ascontiguousarray(Q[i].T)
        m["XkT"] = np.ascontiguousarray(K[i].T)
        m["XvT"] = np.ascontiguousarray(V[i].T.astype(bf))
        m["Qm"] = np.ascontiguousarray(np.asarray(Q_mask[i], np.float32))
        m["Km"] = np.ascontiguousarray(np.asarray(K_mask[i], np.float32))
        in_maps.append(m)
    res = run_bass_kernel_spmd(nc, in_maps, list(range(B)))
    return np.stack([res.results[i]["out"] for i in range(B)], axis=0)


# revision 12
# speedup vs baseline: 1.5896x; 1.1207x over previous
"""Trainium2 Bass kernel for nn_MultiHeadAttention (B=8, S=1024, HID=1024, NH=16).

Data-parallel over batch across 8 NeuronCores (1 batch element/core).
Host prep: weights pre-transposed (WqT pre-scaled by 8, bq by 8), activations
pre-transposed; Q/K path fp32r, V/O path bf16.

Per-core pipeline (single Tile program):
  1. qT/kT = WT @ XT as fp32r matmuls (1 cyc/row); bias added via ACT
     Identity (per-partition AP bias) during PSUM->SBUF copy; output f32r.
  2. v natural [seq,feat] via bf16 matmuls + K=1 bias row; masked by K_mask
     and augmented with [K_mask, 1] columns -> VM bf16.
  3. Per (head, q-block): scores psum [128,1024] via 2 fp32r matmuls (K=64);
     DVE rowmax (negated); ACT exp(s-max) -> e bf16 SBUF; DMA-xbar transpose
     e -> ET chunk (no PE, no DVE).
  4. ctx^T psum [66,512] = sum_kb VM' @ ET; rows 64/65 = U = sum(e*Km),
     D = sum(e).
  5. scale = Q_mask/(U + S*1e-8*D) batched; ctx *= scale; out = ctx@WoT+bo.
"""

import numpy as np
import ml_dtypes

import concourse.bass as bass
import concourse.tile as tile
from concourse import bacc, mybir
from concourse.bass_utils import run_bass_kernel_spmd

F32 = mybir.dt.float32
F32R = mybir.dt.float32r
BF16 = mybir.dt.bfloat16
AF = mybir.ActivationFunctionType
ALU = mybir.AluOpType
AX = mybir.AxisListType

B, S, HID, NH, DH = 8, 1024, 1024, 16, 64
P = 128
NP = HID // P        # 8 hid partition-blocks
NSB = S // P         # 8 seq blocks
NPAIR = NH // 2
CREN = float(S) * 1e-8

_CACHE = {}


def _build():
    nc = bacc.Bacc("TRN2", target_bir_lowering=False, debug=False, num_devices=B)

    XqT = nc.dram_tensor("XqT", [HID, S], F32R, kind="ExternalInput").ap()
    XkT = nc.dram_tensor("XkT", [HID, S], F32R, kind="ExternalInput").ap()
    XvT = nc.dram_tensor("XvT", [HID, S], BF16, kind="ExternalInput").ap()
    Qms = nc.dram_tensor("Qms", [32, 512], F32, kind="ExternalInput").ap()
    Km = nc.dram_tensor("Km", [S], F32, kind="ExternalInput").ap()
    WqT = nc.dram_tensor("WqT", [HID, HID], F32R, kind="ExternalInput").ap()
    WkT = nc.dram_tensor("WkT", [HID, HID], F32R, kind="ExternalInput").ap()
    WvT = nc.dram_tensor("WvT", [HID, HID], BF16, kind="ExternalInput").ap()
    WoT = nc.dram_tensor("WoT", [HID, HID], BF16, kind="ExternalInput").ap()
    bq8 = nc.dram_tensor("bq8", [HID], F32, kind="ExternalInput").ap()
    bkv = nc.dram_tensor("bk", [HID], F32, kind="ExternalInput").ap()
    bvb = nc.dram_tensor("bvb", [HID], BF16, kind="ExternalInput").ap()
    bob = nc.dram_tensor("bob", [HID], BF16, kind="ExternalInput").ap()
    out = nc.dram_tensor("out", [S, HID], F32, kind="ExternalOutput").ap()
    scl_dram = nc.dram_tensor("scl_scratch", [32, 512], F32).ap()

    with tile.TileContext(nc) as tc:
        with (
            tc.tile_pool(name="consts", bufs=1) as consts,
            tc.tile_pool(name="xh", bufs=2) as xh,          # X^T halves
            tc.tile_pool(name="wch", bufs=2) as wch,        # Wq/Wk^T ob-chunks
            tc.tile_pool(name="wbf", bufs=1) as wbf,        # WvT -> WoT bf16
            tc.tile_pool(name="qk", bufs=1) as qkp,
            tc.tile_pool(name="vm", bufs=1) as vmp,
            tc.tile_pool(name="et", bufs=2) as etp,
            tc.tile_pool(name="epool", bufs=2) as epool,
            tc.tile_pool(name="ctx", bufs=1) as ctxp,
            tc.tile_pool(name="smalls", bufs=4) as smalls,
            tc.tile_pool(name="ud", bufs=2) as udp,
            tc.tile_pool(name="pf512", bufs=2) as pf512,
            tc.tile_pool(name="ps_big", bufs=3, space="PSUM") as ps_big,
            tc.tile_pool(name="ps_mid", bufs=2, space="PSUM") as ps_mid,
        ):
            # ---------------- constants ----------------
            km_pi = consts.tile([P, NSB], F32, name="km_pi")
            nc.sync.dma_start(km_pi[:], Km.rearrange("(o p) -> p o", p=P))
            bq8p = consts.tile([P, NP], F32, name="bq8p")
            nc.sync.dma_start(bq8p[:], bq8.rearrange("(o p) -> p o", p=P))
            bkp = consts.tile([P, NP], F32, name="bkp")
            nc.sync.dma_start(bkp[:], bkv.rearrange("(o p) -> p o", p=P))
            onesb = consts.tile([1, P], BF16, name="onesb")
            nc.vector.memset(onesb[:], 1.0)
            bvrow = consts.tile([1, HID], BF16, name="bvrow")
            nc.sync.dma_start(bvrow[:], bvb[None, :])
            borow = consts.tile([1, HID], BF16, name="borow")
            nc.sync.dma_start(borow[:], bob[None, :])
            ST = consts.tile([32, S], F32, name="ST")

            # ---------------- q/k projections (fp32r) ----------------
            qt = qkp.tile([P, NP, S], F32R, name="qt")
            kt = qkp.tile([P, NP, S], F32R, name="kt")

            def qk_projection(x_dram, w_dram, biasp, dst):
                xhalves = []
                for sc in range(2):
                    xt = xh.tile([P, NP, 512], F32R, tag="xh")
                    nc.sync.dma_start(
                        xt[:],
                        x_dram.rearrange("(m p) s -> p m s", p=P)
                        [:, :, sc * 512:(sc + 1) * 512])
                    xhalves.append(xt)
                for ob in range(NP):
                    wc = wch.tile([P, NP, P], F32R, tag="wch")
                    nc.sync.dma_start(
                        wc[:],
                        w_dram.rearrange("(m p) f -> p m f", p=P)
                        [:, :, ob * P:(ob + 1) * P])
                    for sc in range(2):
                        pp = ps_mid.tile([P, 512], F32, tag="mid")
                        for m in range(NP):
                            nc.tensor.matmul(pp[:], wc[:, m, :],
                                             xhalves[sc][:, m, :],
                                             start=(m == 0), stop=(m == NP - 1))
                        nc.scalar.activation(
                            dst[:, ob, sc * 512:(sc + 1) * 512], pp[:],
                            AF.Identity, bias=biasp[:, ob:ob + 1], scale=1.0)

            qk_projection(XqT, WqT, bq8p, qt)
            qk_projection(XkT, WkT, bkp, kt)

            # ---------------- v projection (bf16, natural layout) ----------
            VM = vmp.tile([P, NSB, NH, 66], BF16, name="VM")
            wvt = wbf.tile([P, NP, HID], BF16, tag="wbf")
            nc.sync.dma_start(wvt[:], WvT.rearrange("(m p) f -> p m f", p=P))
            for sc in range(2):
                xvt = xh.tile([P, NP, 512], BF16, tag="xh")
                nc.sync.dma_start(
                    xvt[:],
                    XvT.rearrange("(m p) s -> p m s", p=P)
                    [:, :, sc * 512:(sc + 1) * 512])
                for sb2 in range(4):
                    sb = sc * 4 + sb2
                    for oc in range(2):
                        pp = ps_mid.tile([P, 512], F32, tag="mid")
                        for m in range(NP):
                            nc.tensor.matmul(
                                pp[:], xvt[:, m, sb2 * P:(sb2 + 1) * P],
                                wvt[:, m, oc * 512:(oc + 1) * 512],
                                start=(m == 0), stop=False)
                        nc.tensor.matmul(pp[:], onesb[:],
                                         bvrow[:, oc * 512:(oc + 1) * 512],
                                         start=False, stop=True)
                        nc.vector.tensor_scalar_mul(
                            VM[:, sb, oc * 8:(oc + 1) * 8, 0:64],
                            pp[:].rearrange("p (h d) -> p h d", h=8),
                            km_pi[:, sb:sb + 1])
            for sb in range(NSB):
                nc.vector.tensor_copy(
                    VM[:, sb, :, 64:65],
                    km_pi[:, sb:sb + 1, None].to_broadcast([P, NH, 1]))
            nc.vector.memset(VM[:, :, :, 65:66], 1.0)

            # ---------------- attention (software-pipelined) ----------------
            ctxu = ctxp.tile([P, NPAIR, S], BF16, name="ctxu")

            def emit_ctx(h, qc, et_c):
                j, pb = h // 2, 64 * (h % 2)
                cp = ps_mid.tile([P, 512], F32, tag="mid")
                for kb in range(NSB):
                    nc.tensor.matmul(cp[0:66, :], VM[:, kb, h, :],
                                     et_c[:, :, kb, :],
                                     start=(kb == 0), stop=(kb == NSB - 1))
                idx = h * 2 + qc
                ud = udp.tile([2, 512], F32, tag="ud")
                nc.vector.tensor_copy(ud[:], cp[64:66, :])
                nc.scalar.dma_start(ST[idx:idx + 1, :], ud[:])
                if pb == 0:
                    nc.vector.tensor_copy(
                        ctxu[0:64, j, qc * 512:(qc + 1) * 512], cp[0:64, :])
                else:
                    sg = pf512.tile([64, 512], BF16, tag="pf512")
                    nc.vector.tensor_copy(sg[:], cp[0:64, :])
                    nc.sync.dma_start(
                        ctxu[64:128, j, qc * 512:(qc + 1) * 512], sg[:])

            prev = None
            for h in range(NH):
                j, pb = h // 2, 64 * (h % 2)
                for qc in range(2):
                    # et_c[kp, qb4, kb, q] = e[qb4-block q, kb*128+kp]
                    et_c = etp.tile([P, 4, NSB, P], BF16, tag="etc")
                    e_buf = epool.tile([P, 4, S], BF16, tag="e")
                    for qb4 in range(4):
                        qb = qc * 4 + qb4
                        sp = ps_big.tile([P, S], F32, tag="big")
                        qs = slice(qb * P, (qb + 1) * P)
                        for kc in range(2):
                            ks = slice(kc * 512, (kc + 1) * 512)
                            nc.tensor.matmul(sp[:, ks], qt[pb:pb + 64, j, qs],
                                             kt[pb:pb + 64, j, ks],
                                             start=True, stop=True)
                        nmax = smalls.tile([P, 1], F32, tag="nmax")
                        nc.vector.tensor_reduce(nmax[:], sp[:], axis=AX.X,
                                                op=ALU.max, negate=True)
                        nc.scalar.activation(e_buf[:, qb4, :], sp[:], AF.Exp,
                                             bias=nmax[:], scale=1.0)
                    nc.scalar.dma_start(et_c[:], e_buf[:], transpose=True)
                    if prev is not None:
                        emit_ctx(*prev)
                    prev = (h, qc, et_c)
            emit_ctx(*prev)

            # ---------------- batched renorm ----------------
            qmst = consts.tile([32, 512], F32, name="qmst")
            nc.sync.dma_start(qmst[:], Qms)
            scl = consts.tile([32, 512], F32, name="scl")
            # scl = Qm / (U + CREN * D);  U = ST[:, 0:512], D = ST[:, 512:]
            nc.vector.scalar_tensor_tensor(scl[:], ST[:, 512:1024], CREN,
                                           ST[:, 0:512], ALU.mult, ALU.add)
            nc.vector.reciprocal(scl[:], scl[:])
            nc.vector.tensor_tensor(scl[:], scl[:], qmst[:], ALU.mult)
            nc.sync.dma_start(scl_dram[:], scl[:])

            for j in range(NPAIR):
                for qc in range(2):
                    sd = pf512.tile([P, 512], F32, tag="pf512")
                    ia = (2 * j) * 2 + qc
                    ibx = (2 * j + 1) * 2 + qc
                    nc.sync.dma_start(
                        sd[0:64, :],
                        scl_dram[ia:ia + 1, :].to_broadcast([64, 512]))
                    nc.sync.dma_start(
                        sd[64:128, :],
                        scl_dram[ibx:ibx + 1, :].to_broadcast([64, 512]))
                    nc.vector.tensor_tensor(
                        ctxu[:, j, qc * 512:(qc + 1) * 512],
                        ctxu[:, j, qc * 512:(qc + 1) * 512], sd[:], ALU.mult)

            # ---------------- output projection (bf16) ----------------
            wot = wbf.tile([P, NP, HID], BF16, tag="wbf")
            nc.sync.dma_start(wot[:], WoT.rearrange("(m p) f -> p m f", p=P))
            for qb in range(NSB):
                for oc in range(2):
                    op_ = ps_mid.tile([P, 512], F32, tag="mid")
                    for j in range(NPAIR):
                        nc.tensor.matmul(
                            op_[:], ctxu[:, j, qb * P:(qb + 1) * P],
                            wot[:, j, oc * 512:(oc + 1) * 512],
                            start=(j == 0), stop=False)
                    nc.tensor.matmul(op_[:], onesb[:],
                                     borow[:, oc * 512:(oc + 1) * 512],
                                     start=False, stop=True)
                    ot = pf512.tile([P, 512], F32, tag="pf512")
                    nc.scalar.activation(ot[:], op_[:], AF.Copy)
                    nc.sync.dma_start(
                        out[qb * P:(qb + 1) * P, oc * 512:(oc + 1) * 512], ot[:])

    nc.compile()
    return nc


def kernel(Q, K, V, Q_mask, K_mask, Wq, bq, Wk, bk, Wv, bv, Wo, bo):
    if "nc" not in _CACHE:
        _CACHE["nc"] = _build()
    nc = _CACHE["nc"]
    Q = np.asarray(Q, np.float32)
    K = np.asarray(K, np.float32)
    V = np.asarray(V, np.float32)
    bf = ml_dtypes.bfloat16
    shared = {
        "WqT": np.ascontiguousarray((8.0 * np.asarray(Wq, np.float32)).T),
        "WkT": np.ascontiguousarray(np.asarray(Wk, np.float32).T),
        "WvT": np.ascontiguousarray(np.asarray(Wv, np.float32).T.astype(bf)),
        "WoT": np.ascontiguousarray(np.asarray(Wo, np.float32).T.astype(bf)),
        "bq8": np.ascontiguousarray(8.0 * np.asarray(bq, np.float32)),
        "bk": np.ascontiguousarray(np.asarray(bk, np.float32)),
        "bvb": np.ascontiguousarray(np.asarray(bv, np.float32).astype(bf)),
        "bob": np.ascontiguousarray(np.asarray(bo, np.float32).astype(bf)),
    }
    in_maps = []
    for i in range(B):
        m = dict(shared)
        m["XqT"] = np.ascontiguousarray(Q[i].T)
        m["XkT"] = np.ascontiguousarray(K[i].T)
        m["XvT"] = np.ascontiguousarray(V[i].T.astype(bf))
        qm = np.asarray(Q_mask[i], np.float32)
        m["Qms"] = np.ascontiguousarray(
            np.tile(qm.reshape(2, 512), (16, 1)).reshape(32, 512))
        m["Km"] = np.ascontiguousarray(np.asarray(K_mask[i], np.float32))
        in_maps.append(m)
    res = run_bass_kernel_spmd(nc, in_maps, list(range(B)))
    return np.stack([res.results[i]["out"] for i in range(B)], axis=0)
